# revision 17
# baseline (speedup 1.0000x reference)
"""GATv2 layer kernel for Trainium2, sharded across 8 NeuronCores.

Computation (reference):
    Wh = h @ W.T                       [N, F]
    s1 = Wh @ a1, s2 = Wh @ a2         [N]
    e  = leaky_relu(s1[:,None] + s2[None,:], 0.2)
    attention = softmax(e * adj, dim=1)
    out = attention @ Wh               [N, F]

Sharding: rows (destination nodes) split across 8 cores, 1024 rows each.
Each core gets its adj row-block plus replicated h/W/a, computes its
1024x128 output block; host concatenates.

Per-core pipeline, tiled [128 rows x 2048 cols]:
    ACT : L = Prelu(SJbc + s1_row, alpha=0.2)      (per-partition bias)
    DVE : T = L * adj_tile
    PE  : transpose T 128x128 tiles -> PSUM [128,1024]
    ACT : P^T = Exp(T^T)  PSUM -> SBUF bf16        (fused evacuation)
    PE  : acc += P^T.T @ [Wh | 1] (bf16, FWL)      (ones col = softmax denom)
    DVE : out_rows = acc[:, :128] * 1/acc[:, 128]
Softmax is computed without max subtraction: scores are O(6) so exp is
safely in fp32 range, matching the reference up to fp rounding.
"""
import sys

for _p in ("/opt/trn_rl_repo", "/root/.axon_site/_ro/trn_rl_repo"):
    if _p not in sys.path:
        sys.path.insert(0, _p)

import numpy as np
from contextlib import ExitStack

from concourse import bacc, tile, mybir
from concourse.bass_utils import run_bass_kernel_spmd
from concourse.masks import make_identity

f32 = mybir.dt.float32
bf16 = mybir.dt.bfloat16
AL = mybir.AluOpType
AF = mybir.ActivationFunctionType

N = 8192
F = 128
NCORES = 8
RPC = N // NCORES          # rows per core = 1024
RT = RPC // 128            # row tiles per core = 8
CCH = 2048                 # column chunk for PRELU/MULT
NCH = N // CCH             # col chunks per row tile = 4
PSW = 1024                 # psum transpose tile width (2 banks)
NEG_SLOPE = 0.2

_CACHE = {}


def _build():
    nc = bacc.Bacc("TRN2", target_bir_lowering=False)

    adj_ext = nc.declare_dram_parameter("adj", [RPC, N], f32, isOutput=False)
    hT_ext = nc.declare_dram_parameter("hT", [F, N], f32, isOutput=False)
    hTloc_ext = nc.declare_dram_parameter("hT_loc", [F, RPC], f32, isOutput=False)
    wt_ext = nc.declare_dram_parameter("wt", [F, F], f32, isOutput=False)  # W^T [fi, fo]
    w_ext = nc.declare_dram_parameter("w", [F, F], f32, isOutput=False)    # W [fo, fi]
    a1_ext = nc.declare_dram_parameter("a1", [F, 1], f32, isOutput=False)
    a2_ext = nc.declare_dram_parameter("a2", [F, 1], f32, isOutput=False)
    out_ext = nc.declare_dram_parameter("out", [RPC, F], f32, isOutput=True)

    with tile.TileContext(nc) as tc, ExitStack() as ctx:
        const = ctx.enter_context(tc.tile_pool(name="const", bufs=1))
        setup = ctx.enter_context(tc.tile_pool(name="setup", bufs=3))
        ps_tp = ctx.enter_context(tc.tile_pool(name="ps_tp", bufs=3, space="PSUM"))
        ps_acc = ctx.enter_context(tc.tile_pool(name="ps_acc", bufs=2, space="PSUM"))
        adj_pool = ctx.enter_context(tc.tile_pool(name="adjp", bufs=4))
        work = ctx.enter_context(tc.tile_pool(name="work", bufs=3))
        pexp = ctx.enter_context(tc.tile_pool(name="pexp", bufs=4))
        outp = ctx.enter_context(tc.tile_pool(name="outp", bufs=2))

        ident = const.tile([128, 128], f32)
        make_identity(nc, ident)
        ident_bf = const.tile([128, 128], bf16)
        make_identity(nc, ident_bf)
        wt_sb = const.tile([F, F], f32)
        nc.sync.dma_start(out=wt_sb, in_=wt_ext[:, :])
        w_sb = const.tile([F, F], f32)
        nc.sync.dma_start(out=w_sb, in_=w_ext[:, :])
        a1_sb = const.tile([F, 1], f32)
        nc.sync.dma_start(out=a1_sb, in_=a1_ext[:, :])
        a2_sb = const.tile([F, 1], f32)
        nc.sync.dma_start(out=a2_sb, in_=a2_ext[:, :])
        ones_row = const.tile([1, 128], f32)
        nc.vector.memset(ones_row, 1.0)

        # big persistent tensors, split per column-chunk so the main loop can
        # start on chunk j as soon as its slice of setup is done
        sjbc_t = [const.tile([128, CCH], f32, name=f"sjbc{_}") for _ in range(NCH)]
        whext_t = [const.tile([128, CCH // 128, F + 1], bf16, name=f"whext{_}") for _ in range(NCH)]
        sj_t = [const.tile([1, CCH], f32, name=f"sj{_}") for _ in range(NCH)]
        si_cols = const.tile([128, RT], f32)             # s1 own rows, per-partition

        for jj in range(NCH):
            nc.vector.memset(whext_t[jj][:, :, F:F + 1], 1.0)

        # ---- w1 = W^T a1, w2 = W^T a2 (feature-space vectors) ----
        ps_w = ps_tp.tile([128, PSW], f32, tag="tp")
        nc.tensor.matmul(ps_w[:, 0:1], lhsT=w_sb, rhs=a1_sb, start=True, stop=True)
        nc.tensor.matmul(ps_w[:, 1:2], lhsT=w_sb, rhs=a2_sb, start=True, stop=True)
        w1c = const.tile([128, 1], f32)
        nc.vector.tensor_copy(out=w1c, in_=ps_w[:, 0:1])
        w2c = const.tile([128, 1], f32)
        nc.vector.tensor_copy(out=w2c, in_=ps_w[:, 1:2])

        # ---- own-row s1 column vectors: si = hTloc^T @ w1 ----
        for kk in range(RPC // 512):
            hTlc = setup.tile([128, 512], f32, tag="whT_c")
            nc.sync.dma_start(out=hTlc,
                              in_=hTloc_ext[:, 512 * kk:512 * kk + 512])
            for m in range(4):
                t = 4 * kk + m
                ps3 = ps_acc.tile([128, F + 1], f32, tag="acc")
                nc.tensor.matmul(ps3[:, 0:1],
                                 lhsT=hTlc[:, 128 * m:128 * m + 128],
                                 rhs=w1c, start=True, stop=True)
                nc.vector.tensor_copy(out=si_cols[:, t:t + 1], in_=ps3[:, 0:1])

        # ---- setup and main-loop emitters ----
        def emit_setup_k(k):
            hTc = setup.tile([128, 1024], f32, tag="hTc", name=f"hTc{k}")
            nc.sync.dma_start(out=hTc, in_=hT_ext[:, 1024 * k:1024 * k + 1024])
            for m in range(2):
                ps3 = ps_tp.tile([128, PSW], f32, tag="tp", name=f"psj{k}_{m}")
                nc.tensor.matmul(ps3[0:1, 0:512], lhsT=w2c,
                                 rhs=hTc[:, 512 * m:512 * m + 512],
                                 start=True, stop=True)
                off = 1024 * k + 512 * m
                nc.vector.tensor_copy(
                    out=sj_t[off // CCH][0:1, off % CCH:off % CCH + 512],
                    in_=ps3[0:1, 0:512])
            for m in range(8):
                ps2 = ps_tp.tile([128, PSW], f32, tag="tp", name=f"pwh{k}_{m}")
                nc.tensor.matmul(ps2[:, 0:F], lhsT=hTc[:, 128 * m:128 * m + 128],
                                 rhs=wt_sb, start=True, stop=True)
                ci = 8 * k + m
                nc.vector.tensor_copy(out=whext_t[ci // 16][:, ci % 16, 0:F],
                                      in_=ps2[:, 0:F])
            if k % 2 == 1:
                # broadcast s2 chunk across partitions via K=1 ones matmul
                jj = k // 2
                for m in range(4):
                    ps_b = ps_tp.tile([128, PSW], f32, tag="tp",
                                      name=f"psb{k}_{m}")
                    nc.tensor.matmul(ps_b[:, 0:512], lhsT=ones_row,
                                     rhs=sj_t[jj][0:1, 512 * m:512 * m + 512],
                                     start=True, stop=True)
                    dst = sjbc_t[jj][:, 512 * m:512 * m + 512]
                    if m % 2 == 0:
                        nc.scalar.copy(out=dst, in_=ps_b[:, 0:512])
                    else:
                        nc.vector.tensor_copy(out=dst, in_=ps_b[:, 0:512])

        def emit_main_chunk(t, j, acc):
            adj_t = adj_pool.tile([128, CCH], f32, tag="adj", name=f"adj{t}_{j}")
            nc.sync.dma_start(
                out=adj_t,
                in_=adj_ext[128 * t:128 * t + 128, CCH * j:CCH * j + CCH])
            ch = t * NCH + j
            if (((ch + 1) * 14) // 32) > ((ch * 14) // 32):
                # DVE path: fused (sjbc+si)*adj then leaky via stt
                T0 = work.tile([128, CCH], f32, tag="L", name=f"T0_{t}_{j}")
                nc.vector.scalar_tensor_tensor(
                    out=T0, in0=sjbc_t[j][:, :],
                    scalar=si_cols[:, t:t + 1], in1=adj_t,
                    op0=AL.add, op1=AL.mult)
                T = work.tile([128, CCH], bf16, tag="T", name=f"T_{t}_{j}")
                nc.vector.scalar_tensor_tensor(
                    out=T, in0=T0, scalar=NEG_SLOPE, in1=T0,
                    op0=AL.mult, op1=AL.max)
            else:
                L = work.tile([128, CCH], f32, tag="L", name=f"L_{t}_{j}")
                nc.scalar.activation(out=L, in_=sjbc_t[j][:, :],
                                     func=AF.Prelu, bias=si_cols[:, t:t + 1],
                                     alpha=NEG_SLOPE)
                T = work.tile([128, CCH], bf16, tag="T", name=f"T_{t}_{j}")
                nc.vector.tensor_tensor(out=T, in0=L, in1=adj_t, op=AL.mult)
            for q in range(CCH // PSW):
                tp = ps_tp.tile([128, PSW], bf16, tag="tp", name=f"tp{t}_{j}_{q}")
                for s in range(PSW // 128):
                    nc.tensor.transpose(
                        tp[:, 128 * s:128 * s + 128],
                        T[:, PSW * q + 128 * s:PSW * q + 128 * s + 128],
                        ident_bf)
                P_t = pexp.tile([128, PSW], bf16, tag="P", name=f"P{t}_{j}_{q}")
                nc.scalar.activation(out=P_t, in_=tp, func=AF.Exp)
                for s in range(PSW // 128):
                    ci = (PSW * q + 128 * s) // 128
                    nc.tensor.matmul(
                        acc, lhsT=P_t[:, 128 * s:128 * s + 128],
                        rhs=whext_t[j][:, ci, :],
                        start=(j == 0 and q == 0 and s == 0),
                        stop=(j == NCH - 1 and q == CCH // PSW - 1
                              and s == PSW // 128 - 1))

        def emit_finalize(t, acc):
            rinv = outp.tile([128, 1], f32, tag="rinv", name=f"rinv{t}")
            nc.vector.reciprocal(rinv, acc[:, F:F + 1])
            o_t = outp.tile([128, F], f32, tag="o", name=f"o{t}")
            nc.vector.tensor_scalar(out=o_t, in0=acc[:, 0:F],
                                    scalar1=rinv[:, 0:1], scalar2=None,
                                    op0=AL.mult)
            nc.sync.dma_start(out=out_ext[128 * t:128 * t + 128, :], in_=o_t)

        # interleave setup chunk-pairs with the first row-tile's chunks so
        # the PE stream doesn't serialize all setup before the main loop
        emit_setup_k(0)
        emit_setup_k(1)
        emit_setup_k(2)
        emit_setup_k(3)
        acc0 = ps_acc.tile([128, F + 1], f32, tag="acc", name="acc0")
        emit_main_chunk(0, 0, acc0)
        emit_setup_k(4)
        emit_setup_k(5)
        emit_main_chunk(0, 1, acc0)
        emit_setup_k(6)
        emit_setup_k(7)
        emit_main_chunk(0, 2, acc0)
        emit_main_chunk(0, 3, acc0)
        emit_finalize(0, acc0)
        for t in range(1, RT):
            acc = ps_acc.tile([128, F + 1], f32, tag="acc", name=f"acc{t}")
            for j in range(NCH):
                emit_main_chunk(t, j, acc)
            emit_finalize(t, acc)

    nc.compile()
    return nc


def _get_nc():
    if "nc" not in _CACHE:
        _CACHE["nc"] = _build()
    return _CACHE["nc"]


def kernel(h, adj, W, a, _trace=False, _trace_kwargs=None):
    h = np.ascontiguousarray(np.asarray(h, dtype=np.float32))
    adj = np.ascontiguousarray(np.asarray(adj, dtype=np.float32))
    W = np.asarray(W, dtype=np.float32)
    a = np.asarray(a, dtype=np.float32)

    wt = np.ascontiguousarray(W.T)                    # [fi, fo]
    a1c = np.ascontiguousarray(a[0, :F].reshape(F, 1))
    a2c = np.ascontiguousarray(a[0, F:].reshape(F, 1))

    hT = np.ascontiguousarray(h.T)                    # [fi, n]
    nc = _get_nc()
    in_maps = []
    for c in range(NCORES):
        r0 = c * RPC
        in_maps.append({
            "adj": np.ascontiguousarray(adj[r0:r0 + RPC, :]),
            "hT": hT,
            "hT_loc": np.ascontiguousarray(hT[:, r0:r0 + RPC]),
            "wt": wt,
            "w": W,
            "a1": a1c,
            "a2": a2c,
        })
    kw = {}
    if _trace:
        kw["trace"] = True
        kw.update(_trace_kwargs or {})
    res = run_bass_kernel_spmd(nc, in_maps, core_ids=list(range(NCORES)), **kw)
    out = np.concatenate([res.results[c]["out"] for c in range(NCORES)], axis=0)
    if _trace:
        return out, res
    return out


# revision 18
# speedup vs baseline: 1.0705x; 1.0705x over previous
"""GATv2 layer kernel for Trainium2, sharded across 8 NeuronCores.

Computation (reference):
    Wh = h @ W.T                       [N, F]
    s1 = Wh @ a1, s2 = Wh @ a2         [N]
    e  = leaky_relu(s1[:,None] + s2[None,:], 0.2)
    attention = softmax(e * adj, dim=1)
    out = attention @ Wh               [N, F]

Sharding: rows (destination nodes) split across 8 cores, 1024 rows each.
Each core gets its adj row-block plus replicated h/W/a, computes its
1024x128 output block; host concatenates.

Per-core pipeline, tiled [128 rows x 2048 cols]:
    ACT : L = Prelu(SJbc + s1_row, alpha=0.2)      (per-partition bias)
    DVE : T = L * adj_tile
    PE  : transpose T 128x128 tiles -> PSUM [128,1024]
    ACT : P^T = Exp(T^T)  PSUM -> SBUF bf16        (fused evacuation)
    PE  : acc += P^T.T @ [Wh | 1] (bf16, FWL)      (ones col = softmax denom)
    DVE : out_rows = acc[:, :128] * 1/acc[:, 128]
Softmax is computed without max subtraction: scores are O(6) so exp is
safely in fp32 range, matching the reference up to fp rounding.
"""
import sys

for _p in ("/opt/trn_rl_repo", "/root/.axon_site/_ro/trn_rl_repo"):
    if _p not in sys.path:
        sys.path.insert(0, _p)

import numpy as np
import ml_dtypes
from contextlib import ExitStack

from concourse import bacc, tile, mybir
from concourse.bass_utils import run_bass_kernel_spmd
from concourse.masks import make_identity

f32 = mybir.dt.float32
bf16 = mybir.dt.bfloat16
AL = mybir.AluOpType
AF = mybir.ActivationFunctionType

N = 8192
F = 128
NCORES = 8
RPC = N // NCORES          # rows per core = 1024
RT = RPC // 128            # row tiles per core = 8
CCH = 2048                 # column chunk for PRELU/MULT
NCH = N // CCH             # col chunks per row tile = 4
PSW = 1024                 # psum transpose tile width (2 banks)
NEG_SLOPE = 0.2

_CACHE = {}


def _build():
    nc = bacc.Bacc("TRN2", target_bir_lowering=False)

    adj_ext = nc.declare_dram_parameter("adj", [RPC, N], bf16, isOutput=False)
    hT_ext = nc.declare_dram_parameter("hT", [F, N], f32, isOutput=False)
    hTloc_ext = nc.declare_dram_parameter("hT_loc", [F, RPC], f32, isOutput=False)
    wt_ext = nc.declare_dram_parameter("wt", [F, F], f32, isOutput=False)  # W^T [fi, fo]
    w_ext = nc.declare_dram_parameter("w", [F, F], f32, isOutput=False)    # W [fo, fi]
    a1_ext = nc.declare_dram_parameter("a1", [F, 1], f32, isOutput=False)
    a2_ext = nc.declare_dram_parameter("a2", [F, 1], f32, isOutput=False)
    out_ext = nc.declare_dram_parameter("out", [RPC, F], f32, isOutput=True)

    with tile.TileContext(nc) as tc, ExitStack() as ctx:
        const = ctx.enter_context(tc.tile_pool(name="const", bufs=1))
        setup = ctx.enter_context(tc.tile_pool(name="setup", bufs=3))
        ps_tp = ctx.enter_context(tc.tile_pool(name="ps_tp", bufs=3, space="PSUM"))
        ps_acc = ctx.enter_context(tc.tile_pool(name="ps_acc", bufs=2, space="PSUM"))
        adj_pool = ctx.enter_context(tc.tile_pool(name="adjp", bufs=4))
        work = ctx.enter_context(tc.tile_pool(name="work", bufs=3))
        pexp = ctx.enter_context(tc.tile_pool(name="pexp", bufs=4))
        outp = ctx.enter_context(tc.tile_pool(name="outp", bufs=2))

        ident = const.tile([128, 128], f32)
        make_identity(nc, ident)
        ident_bf = const.tile([128, 128], bf16)
        make_identity(nc, ident_bf)
        wt_sb = const.tile([F, F], f32)
        nc.sync.dma_start(out=wt_sb, in_=wt_ext[:, :])
        w_sb = const.tile([F, F], f32)
        nc.sync.dma_start(out=w_sb, in_=w_ext[:, :])
        a1_sb = const.tile([F, 1], f32)
        nc.sync.dma_start(out=a1_sb, in_=a1_ext[:, :])
        a2_sb = const.tile([F, 1], f32)
        nc.sync.dma_start(out=a2_sb, in_=a2_ext[:, :])
        ones_row = const.tile([1, 128], f32)
        nc.vector.memset(ones_row, 1.0)

        # big persistent tensors, split per column-chunk so the main loop can
        # start on chunk j as soon as its slice of setup is done
        sjbc_t = [const.tile([128, CCH], f32, name=f"sjbc{_}") for _ in range(NCH)]
        whext_t = [const.tile([128, CCH // 128, F + 1], bf16, name=f"whext{_}") for _ in range(NCH)]
        sj_t = [const.tile([1, CCH], f32, name=f"sj{_}") for _ in range(NCH)]
        si_cols = const.tile([128, RT], f32)             # s1 own rows, per-partition

        for jj in range(NCH):
            nc.vector.memset(whext_t[jj][:, :, F:F + 1], 1.0)

        # ---- w1 = W^T a1, w2 = W^T a2 (feature-space vectors) ----
        ps_w = ps_tp.tile([128, PSW], f32, tag="tp")
        nc.tensor.matmul(ps_w[:, 0:1], lhsT=w_sb, rhs=a1_sb, start=True, stop=True)
        nc.tensor.matmul(ps_w[:, 1:2], lhsT=w_sb, rhs=a2_sb, start=True, stop=True)
        w1c = const.tile([128, 1], f32)
        nc.vector.tensor_copy(out=w1c, in_=ps_w[:, 0:1])
        w2c = const.tile([128, 1], f32)
        nc.vector.tensor_copy(out=w2c, in_=ps_w[:, 1:2])

        # ---- own-row s1 column vectors: si = hTloc^T @ w1 ----
        for kk in range(RPC // 512):
            hTlc = setup.tile([128, 512], f32, tag="whT_c")
            nc.sync.dma_start(out=hTlc,
                              in_=hTloc_ext[:, 512 * kk:512 * kk + 512])
            for m in range(4):
                t = 4 * kk + m
                ps3 = ps_acc.tile([128, F + 1], f32, tag="acc")
                nc.tensor.matmul(ps3[:, 0:1],
                                 lhsT=hTlc[:, 128 * m:128 * m + 128],
                                 rhs=w1c, start=True, stop=True)
                nc.vector.tensor_copy(out=si_cols[:, t:t + 1], in_=ps3[:, 0:1])

        # ---- setup and main-loop emitters ----
        def emit_setup_k(k):
            hTc = setup.tile([128, 1024], f32, tag="hTc", name=f"hTc{k}")
            nc.sync.dma_start(out=hTc, in_=hT_ext[:, 1024 * k:1024 * k + 1024])
            for m in range(2):
                ps3 = ps_tp.tile([128, PSW], f32, tag="tp", name=f"psj{k}_{m}")
                nc.tensor.matmul(ps3[0:1, 0:512], lhsT=w2c,
                                 rhs=hTc[:, 512 * m:512 * m + 512],
                                 start=True, stop=True)
                off = 1024 * k + 512 * m
                nc.vector.tensor_copy(
                    out=sj_t[off // CCH][0:1, off % CCH:off % CCH + 512],
                    in_=ps3[0:1, 0:512])
            for m in range(8):
                ps2 = ps_tp.tile([128, PSW], f32, tag="tp", name=f"pwh{k}_{m}")
                nc.tensor.matmul(ps2[:, 0:F], lhsT=hTc[:, 128 * m:128 * m + 128],
                                 rhs=wt_sb, start=True, stop=True)
                ci = 8 * k + m
                nc.vector.tensor_copy(out=whext_t[ci // 16][:, ci % 16, 0:F],
                                      in_=ps2[:, 0:F])
            if k % 2 == 1:
                # broadcast s2 chunk across partitions via K=1 ones matmul
                jj = k // 2
                for m in range(4):
                    ps_b = ps_tp.tile([128, PSW], f32, tag="tp",
                                      name=f"psb{k}_{m}")
                    nc.tensor.matmul(ps_b[:, 0:512], lhsT=ones_row,
                                     rhs=sj_t[jj][0:1, 512 * m:512 * m + 512],
                                     start=True, stop=True)
                    dst = sjbc_t[jj][:, 512 * m:512 * m + 512]
                    if m % 2 == 0:
                        nc.scalar.copy(out=dst, in_=ps_b[:, 0:512])
                    else:
                        nc.vector.tensor_copy(out=dst, in_=ps_b[:, 0:512])

        def emit_main_chunk(t, j, acc):
            adj_t = adj_pool.tile([128, CCH], bf16, tag="adj", name=f"adj{t}_{j}")
            nc.sync.dma_start(
                out=adj_t,
                in_=adj_ext[128 * t:128 * t + 128, CCH * j:CCH * j + CCH])
            ch = t * NCH + j
            if (((ch + 1) * 15) // 32) > ((ch * 15) // 32):
                # DVE path: fused (sjbc+si)*adj then leaky via stt
                T0 = work.tile([128, CCH], bf16, tag="Lb", name=f"T0_{t}_{j}")
                nc.vector.scalar_tensor_tensor(
                    out=T0, in0=sjbc_t[j][:, :],
                    scalar=si_cols[:, t:t + 1], in1=adj_t,
                    op0=AL.add, op1=AL.mult)
                T = work.tile([128, CCH], bf16, tag="T", name=f"T_{t}_{j}")
                nc.vector.scalar_tensor_tensor(
                    out=T, in0=T0, scalar=NEG_SLOPE, in1=T0,
                    op0=AL.mult, op1=AL.max)
            else:
                L = work.tile([128, CCH], bf16, tag="Lb", name=f"L_{t}_{j}")
                nc.scalar.activation(out=L, in_=sjbc_t[j][:, :],
                                     func=AF.Prelu, bias=si_cols[:, t:t + 1],
                                     alpha=NEG_SLOPE)
                T = work.tile([128, CCH], bf16, tag="T", name=f"T_{t}_{j}")
                nc.vector.tensor_tensor(out=T, in0=L, in1=adj_t, op=AL.mult)
            for q in range(CCH // PSW):
                tp = ps_tp.tile([128, PSW], bf16, tag="tp", name=f"tp{t}_{j}_{q}")
                for s in range(PSW // 128):
                    nc.tensor.transpose(
                        tp[:, 128 * s:128 * s + 128],
                        T[:, PSW * q + 128 * s:PSW * q + 128 * s + 128],
                        ident_bf)
                P_t = pexp.tile([128, PSW], bf16, tag="P", name=f"P{t}_{j}_{q}")
                nc.scalar.activation(out=P_t, in_=tp, func=AF.Exp)
                for s in range(PSW // 128):
                    ci = (PSW * q + 128 * s) // 128
                    nc.tensor.matmul(
                        acc, lhsT=P_t[:, 128 * s:128 * s + 128],
                        rhs=whext_t[j][:, ci, :],
                        start=(j == 0 and q == 0 and s == 0),
                        stop=(j == NCH - 1 and q == CCH // PSW - 1
                              and s == PSW // 128 - 1))

        def emit_finalize(t, acc):
            rinv = outp.tile([128, 1], f32, tag="rinv", name=f"rinv{t}")
            nc.vector.reciprocal(rinv, acc[:, F:F + 1])
            o_t = outp.tile([128, F], f32, tag="o", name=f"o{t}")
            nc.vector.tensor_scalar(out=o_t, in0=acc[:, 0:F],
                                    scalar1=rinv[:, 0:1], scalar2=None,
                                    op0=AL.mult)
            nc.sync.dma_start(out=out_ext[128 * t:128 * t + 128, :], in_=o_t)

        # interleave setup chunk-pairs with the first row-tile's chunks so
        # the PE stream doesn't serialize all setup before the main loop
        emit_setup_k(0)
        emit_setup_k(1)
        emit_setup_k(2)
        emit_setup_k(3)
        acc0 = ps_acc.tile([128, F + 1], f32, tag="acc", name="acc0")
        emit_main_chunk(0, 0, acc0)
        emit_setup_k(4)
        emit_setup_k(5)
        emit_main_chunk(0, 1, acc0)
        emit_setup_k(6)
        emit_setup_k(7)
        emit_main_chunk(0, 2, acc0)
        emit_main_chunk(0, 3, acc0)
        emit_finalize(0, acc0)
        for t in range(1, RT):
            acc = ps_acc.tile([128, F + 1], f32, tag="acc", name=f"acc{t}")
            for j in range(NCH):
                emit_main_chunk(t, j, acc)
            emit_finalize(t, acc)

    nc.compile()
    return nc


def _get_nc():
    if "nc" not in _CACHE:
        _CACHE["nc"] = _build()
    return _CACHE["nc"]


def kernel(h, adj, W, a, _trace=False, _trace_kwargs=None):
    h = np.ascontiguousarray(np.asarray(h, dtype=np.float32))
    adj = np.ascontiguousarray(np.asarray(adj, dtype=np.float32))
    W = np.asarray(W, dtype=np.float32)
    a = np.asarray(a, dtype=np.float32)

    wt = np.ascontiguousarray(W.T)                    # [fi, fo]
    a1c = np.ascontiguousarray(a[0, :F].reshape(F, 1))
    a2c = np.ascontiguousarray(a[0, F:].reshape(F, 1))

    hT = np.ascontiguousarray(h.T)                    # [fi, n]
    nc = _get_nc()
    in_maps = []
    for c in range(NCORES):
        r0 = c * RPC
        in_maps.append({
            "adj": np.ascontiguousarray(
                adj[r0:r0 + RPC, :].astype(ml_dtypes.bfloat16)),
            "hT": hT,
            "hT_loc": np.ascontiguousarray(hT[:, r0:r0 + RPC]),
            "wt": wt,
            "w": W,
            "a1": a1c,
            "a2": a2c,
        })
    kw = {}
    if _trace:
        kw["trace"] = True
        kw.update(_trace_kwargs or {})
    res = run_bass_kernel_spmd(nc, in_maps, core_ids=list(range(NCORES)), **kw)
    out = np.concatenate([res.results[c]["out"] for c in range(NCORES)], axis=0)
    if _trace:
        return out, res
    return out


# revision 22
# speedup vs baseline: 1.1115x; 1.0383x over previous
"""GATv2 layer kernel for Trainium2, sharded across 8 NeuronCores.

Computation (reference):
    Wh = h @ W.T                       [N, F]
    s1 = Wh @ a1, s2 = Wh @ a2         [N]
    e  = leaky_relu(s1[:,None] + s2[None,:], 0.2)
    attention = softmax(e * adj, dim=1)
    out = attention @ Wh               [N, F]

Sharding: rows (destination nodes) split across 8 cores, 1024 rows each.
Each core gets its adj row-block plus replicated h/W/a, computes its
1024x128 output block; host concatenates.

adj is 0/1-valued so the host casts it to bf16 losslessly; this halves the
HBM stream AND enables the DMA xbar transpose (2-byte dtypes only), which
delivers adj^T tiles [c, r] directly. The whole pipeline then runs in
transposed layout and the PE never transposes anything:

    per column-chunk ci (128 source nodes x all 1024 own rows):
      DMA : adjT = transpose-DMA adj[:, ci-block]          [128c, 1024r]
      ACT : L = Prelu(SIbc + s2_col[ci], 0.2)  (bias = per-partition s2)
            (or a fused DVE stt pair, load-balanced via PHI)
      DVE : T = L * adjT        (bf16 2x mode)
      ACT : P = Exp(T)          (bf16, sbuf->sbuf, 2-chunk batches)
      PE  : acc[t] += P[:, t-slice].T @ [Wh | 1]   for the 8 row-tiles
    finalize: out_rows[t] = acc[t][:, :128] / acc[t][:, 128]

Softmax runs without max subtraction: scores are O(6) so exp stays in
fp32 range; matches the reference up to fp rounding.
"""
import sys

for _p in ("/opt/trn_rl_repo", "/root/.axon_site/_ro/trn_rl_repo"):
    if _p not in sys.path:
        sys.path.insert(0, _p)

import numpy as np
import ml_dtypes
from contextlib import ExitStack

from concourse import bacc, tile, mybir
from concourse.bass_utils import run_bass_kernel_spmd

f32 = mybir.dt.float32
bf16 = mybir.dt.bfloat16
AL = mybir.AluOpType
AF = mybir.ActivationFunctionType

N = 8192
F = 128
NCORES = 8
RPC = N // NCORES          # rows per core = 1024
RT = RPC // 128            # row tiles per core = 8
NCI = N // 128             # column chunks = 64
PHI = 24                   # of 64 chunks routed to the DVE leaky path
NEG_SLOPE = 0.2

_CACHE = {}


def _build():
    nc = bacc.Bacc("TRN2", target_bir_lowering=False)

    adj_ext = nc.declare_dram_parameter("adj", [RPC, N], bf16, isOutput=False)
    hT_ext = nc.declare_dram_parameter("hT", [F, N], f32, isOutput=False)
    hTloc_ext = nc.declare_dram_parameter("hT_loc", [F, RPC], f32, isOutput=False)
    wt_ext = nc.declare_dram_parameter("wt", [F, F], f32, isOutput=False)  # W^T
    w_ext = nc.declare_dram_parameter("w", [F, F], f32, isOutput=False)    # W
    a1_ext = nc.declare_dram_parameter("a1", [F, 1], f32, isOutput=False)
    a2_ext = nc.declare_dram_parameter("a2", [F, 1], f32, isOutput=False)
    out_ext = nc.declare_dram_parameter("out", [RPC, F], f32, isOutput=True)

    with tile.TileContext(nc) as tc, ExitStack() as ctx:
        const = ctx.enter_context(tc.tile_pool(name="const", bufs=1))
        setup = ctx.enter_context(tc.tile_pool(name="setup", bufs=3))
        psum = ctx.enter_context(tc.tile_pool(name="psum", bufs=8, space="PSUM"))
        adj_pool = ctx.enter_context(tc.tile_pool(name="adjp", bufs=6))
        work = ctx.enter_context(tc.tile_pool(name="work", bufs=3))
        pexp = ctx.enter_context(tc.tile_pool(name="pexp", bufs=3))
        outp = ctx.enter_context(tc.tile_pool(name="outp", bufs=2))

        wt_sb = const.tile([F, F], f32)
        nc.sync.dma_start(out=wt_sb, in_=wt_ext[:, :])
        w_sb = const.tile([F, F], f32)
        nc.sync.dma_start(out=w_sb, in_=w_ext[:, :])
        a1_sb = const.tile([F, 1], f32)
        nc.sync.dma_start(out=a1_sb, in_=a1_ext[:, :])
        a2_sb = const.tile([F, 1], f32)
        nc.sync.dma_start(out=a2_sb, in_=a2_ext[:, :])
        ones_row = const.tile([1, 128], f32)
        nc.vector.memset(ones_row, 1.0)

        # persistent tensors
        whext_t = [const.tile([128, NCI // 4, F + 1], bf16, name=f"whext{_}")
                   for _ in range(4)]
        sj_cols = const.tile([128, NCI], f32)     # s2, column layout
        sibc = const.tile([128, RPC], f32)        # s1 own rows, bcast over parts
        for jj in range(4):
            nc.vector.memset(whext_t[jj][:, :, F:F + 1], 1.0)

        # w1 = W^T a1, w2 = W^T a2 ; wt2 = [W^T | w2]
        ps_w = psum.tile([128, 512], f32, tag="acc")
        nc.tensor.matmul(ps_w[:, 0:1], lhsT=w_sb, rhs=a1_sb, start=True, stop=True)
        nc.tensor.matmul(ps_w[:, 1:2], lhsT=w_sb, rhs=a2_sb, start=True, stop=True)
        w1c = const.tile([128, 1], f32)
        nc.vector.tensor_copy(out=w1c, in_=ps_w[:, 0:1])
        wt2_sb = const.tile([F, F + 1], f32)
        nc.vector.tensor_copy(out=wt2_sb[:, 0:F], in_=wt_sb)
        nc.vector.tensor_copy(out=wt2_sb[:, F:F + 1], in_=ps_w[:, 1:2])

        # s1 own rows -> free layout -> broadcast across partitions
        si_sb = const.tile([1, RPC], f32)
        for kk in range(RPC // 512):
            hTlc = setup.tile([128, 512], f32, tag="hTlc", name=f"hTlc{kk}")
            nc.sync.dma_start(out=hTlc,
                              in_=hTloc_ext[:, 512 * kk:512 * kk + 512])
            ps_si = psum.tile([128, 512], f32, tag="acc", name=f"psi{kk}")
            nc.tensor.matmul(ps_si[0:1, 0:512], lhsT=w1c, rhs=hTlc,
                             start=True, stop=True)
            nc.vector.tensor_copy(out=si_sb[0:1, 512 * kk:512 * kk + 512],
                                  in_=ps_si[0:1, 0:512])
        for kk in range(RPC // 512):
            ps_sib = psum.tile([128, 512], f32, tag="acc", name=f"psib{kk}")
            nc.tensor.matmul(ps_sib[:, 0:512], lhsT=ones_row,
                             rhs=si_sb[0:1, 512 * kk:512 * kk + 512],
                             start=True, stop=True)
            nc.scalar.copy(out=sibc[:, 512 * kk:512 * kk + 512],
                           in_=ps_sib[:, 0:512])

        # stream hT chunks: whext tiles (Wh | 1) and s2 columns via [W^T | w2]
        def emit_setup_k(k):
            hTc = setup.tile([128, 1024], f32, tag="hTc", name=f"hTc{k}")
            nc.sync.dma_start(out=hTc, in_=hT_ext[:, 1024 * k:1024 * k + 1024])
            for m in range(8):
                ci = 8 * k + m
                ps2 = psum.tile([128, 512], f32, tag="acc", name=f"pwh{ci}")
                nc.tensor.matmul(ps2[:, 0:F + 1],
                                 lhsT=hTc[:, 128 * m:128 * m + 128],
                                 rhs=wt2_sb, start=True, stop=True)
                nc.vector.tensor_copy(out=whext_t[ci // 16][:, ci % 16, 0:F],
                                      in_=ps2[:, 0:F])
                nc.vector.tensor_copy(out=sj_cols[:, ci:ci + 1],
                                      in_=ps2[:, F:F + 1])

        # main chunk: 128 source nodes x all own rows
        def emit_main_ci(ci, accs, pair_buf):
            adjT = adj_pool.tile([128, RPC], bf16, tag="adjT", name=f"adjT{ci}")
            nc.sync.dma_start_transpose(adjT,
                                        adj_ext[:, 128 * ci:128 * ci + 128])
            q = ci % 2
            if (((ci + 1) * PHI) // NCI) > ((ci * PHI) // NCI):
                # DVE path: fused (sibc + s2)*adjT, then leaky via stt
                T0 = work.tile([128, RPC], bf16, tag="T0", name=f"T0_{ci}")
                nc.vector.scalar_tensor_tensor(
                    out=T0, in0=sibc, scalar=sj_cols[:, ci:ci + 1], in1=adjT,
                    op0=AL.add, op1=AL.mult)
                nc.vector.scalar_tensor_tensor(
                    out=pair_buf[:, RPC * q:RPC * q + RPC], in0=T0,
                    scalar=NEG_SLOPE, in1=T0, op0=AL.mult, op1=AL.max)
            else:
                L = work.tile([128, RPC], bf16, tag="L", name=f"L_{ci}")
                nc.scalar.activation(out=L, in_=sibc, func=AF.Prelu,
                                     bias=sj_cols[:, ci:ci + 1],
                                     alpha=NEG_SLOPE)
                nc.vector.tensor_tensor(out=pair_buf[:, RPC * q:RPC * q + RPC],
                                        in0=L, in1=adjT, op=AL.mult)
            if q == 1:
                P2 = pexp.tile([128, 2 * RPC], bf16, tag="P", name=f"P{ci}")
                nc.scalar.activation(out=P2, in_=pair_buf, func=AF.Exp)
                for h in range(2):
                    cih = ci - 1 + h
                    for t in range(RT):
                        nc.tensor.matmul(
                            accs[t],
                            lhsT=P2[:, RPC * h + 128 * t:RPC * h + 128 * t + 128],
                            rhs=whext_t[cih // 16][:, cih % 16, :],
                            start=(cih == 0), stop=(cih == NCI - 1),
                            skip_group_check=True)

        accs = [psum.tile([128, 512], f32, tag="acc", name=f"acc{t}")[:, 0:F + 1]
                for t in range(RT)]

        for k in range(8):
            emit_setup_k(k)
        pair = None
        for ci_iter in range(NCI):
            if ci_iter % 2 == 0:
                pair = work.tile([128, 2 * RPC], bf16, tag="T",
                                 name=f"Tp{ci_iter}")
            emit_main_ci(ci_iter, accs, pair)

        for t in range(RT):
            rinv = outp.tile([128, 1], f32, tag="rinv", name=f"rinv{t}")
            nc.vector.reciprocal(rinv, accs[t][:, F:F + 1])
            o_t = outp.tile([128, F], f32, tag="o", name=f"o{t}")
            nc.vector.tensor_scalar(out=o_t, in0=accs[t][:, 0:F],
                                    scalar1=rinv[:, 0:1], scalar2=None,
                                    op0=AL.mult)
            nc.sync.dma_start(out=out_ext[128 * t:128 * t + 128, :], in_=o_t)

    nc.compile()
    return nc


def _get_nc():
    if "nc" not in _CACHE:
        _CACHE["nc"] = _build()
    return _CACHE["nc"]


def kernel(h, adj, W, a, _trace=False, _trace_kwargs=None):
    h = np.ascontiguousarray(np.asarray(h, dtype=np.float32))
    adj = np.asarray(adj, dtype=np.float32)
    W = np.asarray(W, dtype=np.float32)
    a = np.asarray(a, dtype=np.float32)

    wt = np.ascontiguousarray(W.T)                    # [fi, fo]
    a1c = np.ascontiguousarray(a[0, :F].reshape(F, 1))
    a2c = np.ascontiguousarray(a[0, F:].reshape(F, 1))
    hT = np.ascontiguousarray(h.T)                    # [fi, n]
    adj_bf = adj.astype(ml_dtypes.bfloat16)           # 0/1 values: lossless

    nc = _get_nc()
    in_maps = []
    for c in range(NCORES):
        r0 = c * RPC
        in_maps.append({
            "adj": np.ascontiguousarray(adj_bf[r0:r0 + RPC, :]),
            "hT": hT,
            "hT_loc": np.ascontiguousarray(hT[:, r0:r0 + RPC]),
            "wt": wt,
            "w": W,
            "a1": a1c,
            "a2": a2c,
        })
    kw = {}
    if _trace:
        kw["trace"] = True
        kw.update(_trace_kwargs or {})
    res = run_bass_kernel_spmd(nc, in_maps, core_ids=list(range(NCORES)), **kw)
    out = np.concatenate([res.results[c]["out"] for c in range(NCORES)], axis=0)
    if _trace:
        return out, res
    return out
